# revision 27
# baseline (speedup 1.0000x reference)
"""Multi-head attention (relu + valid-key-count normalization) on 8 TRN2 cores.

Strategy: data-parallel over batch (B=16 -> 2 per core), no collectives.
All matmul operands are float16 (full PE rate; 11-bit mantissa keeps the
end-to-end rel err ~1e-3 against the 2e-2 gate).

Key transformations vs the v1 baseline (367.7us):
- The reference normalization collapses to
      A_final[q,k] = relu(A[q,k]) * mask[q,k] * scale / max(m[q],1),
  and the whole multiplicative factor maskq[k,q] = mask[q,k]*scale/max(m,1)
  is precomputed ON HOST as an f16 tensor (transposed to match the
  k-on-partitions logits layout). This removes the on-device mask casts,
  mask transposes (8,192 PE cyc/batch), the m[q] accumulation, the qs
  broadcast chain, and the per-head Q'-scale multiply. relu+mask+norm is
  a single DVE scalar_tensor_tensor per logits tile, straight from PSUM.
- V^T is computed directly on the PE as x-stationary matmuls
  (V^T[k,c] = sum_u x[u,k] wv[u,c]) instead of V followed by PE
  transposes: same matmul cycles, minus 8,192 transpose cyc/batch.
- Weights are loaded once per core (f16, host-packed per head so every
  DMA is a full-contiguous descriptor) and stay resident in SBUF for
  both batches; x/mask DMAs are f16 (half the bytes of v1).
- A short chain of id16 warm-up transposes starts the PE p-state ramp
  during the prologue DMAs, so the first real matmuls run at full clock.

PE work per core: 786,432 cycles @2.4GHz = 327.7us floor; the rest is
prologue DMA, the drain tail, and scheduling gaps.
"""
import sys

sys.path.insert(0, "/opt/trn_rl_repo")

import numpy as np

import concourse.bacc as bacc
import concourse.mybir as mybir
import concourse.tile as tile
from concourse.bass_utils import run_bass_kernel_spmd

B, U, S, H, C = 16, 1024, 1024, 8, 128
NCORES = 8
BPC = B // NCORES  # batches per core
SCALE = float(1.0 / np.sqrt(np.float32(C)))
P = 128  # partitions
UC = U // P  # u chunks
QT = S // P  # q tiles
KT = S // P  # k tiles
NH = 512  # matmul free dim (one PSUM bank of f32)
NWARM = 27  # p-state warm-up transposes: bridge PE busy ~1us -> first matmul

F32 = mybir.dt.float32
F16 = mybir.dt.float16


def build():
    nc = bacc.Bacc()
    x_d = nc.dram_tensor("x16", [BPC, UC, P, S], F16, kind="ExternalInput")
    mq_d = nc.dram_tensor("maskq", [BPC, KT, P, S], F16, kind="ExternalInput")
    wq_d = nc.dram_tensor("wq", [H, P, UC, C], F16, kind="ExternalInput")
    wk_d = nc.dram_tensor("wk", [H, P, UC, C], F16, kind="ExternalInput")
    wv_d = nc.dram_tensor("wv", [H, P, UC, C], F16, kind="ExternalInput")
    wo_d = nc.dram_tensor("wo", [P, UC, U], F16, kind="ExternalInput")
    out_d = nc.dram_tensor("out", [BPC, U, S], F32, kind="ExternalOutput")

    xv = x_d[:].rearrange("b u p s -> b p u s")
    mqv = mq_d[:].rearrange("b k p s -> b p k s")

    with tile.TileContext(nc) as tc:
        with (
            tc.tile_pool(name="sb", bufs=1) as sb,
            tc.tile_pool(name="ps", bufs=1, space="PSUM") as ps,
        ):
            # warm-up: a DVE memset (no DMA dependency) feeds a chain of PE
            # transposes that keep the PE continuously busy from ~1us until
            # the first real matmul — otherwise the p-state ramp restarts
            # after the prologue idle and the first ~6us of real matmuls run
            # at 0.65-1.2GHz instead of 2.4GHz.
            wsrc = sb.tile([P, P], F16, tag="wsrc")
            nc.vector.memset(wsrc[:], 0.0)
            warm = ps.tile([P, P], F16, tag="a", bufs=2, name="warm")
            for _ in range(NWARM):
                nc.tensor.transpose(warm[:], wsrc[:], wsrc[:])

            # resident weights; head 0 first so QKV can start ASAP
            wq_sb = [
                sb.tile([P, UC, C], F16, tag=f"wq{h}", name=f"wq_sb{h}")
                for h in range(H)
            ]
            wk_sb = [
                sb.tile([P, UC, C], F16, tag=f"wk{h}", name=f"wk_sb{h}")
                for h in range(H)
            ]
            wv_sb = [
                sb.tile([P, UC, C], F16, tag=f"wv{h}", name=f"wv_sb{h}")
                for h in range(H)
            ]
            x_ch = [
                [
                    sb.tile([P, S], F16, tag=f"x{uc}", bufs=2, name=f"x{b}_{uc}")
                    for uc in range(UC)
                ]
                for b in range(BPC)
            ]
            mq_sb = [
                sb.tile([P, KT, S], F16, tag="mq", bufs=2, name=f"mq{b}")
                for b in range(BPC)
            ]
            # ALL loads on the SP queue in strict priority order — the DMA
            # engines device is serialized, so transfer order IS this order.
            # batch-0 x arrives as per-uc column-half chunks so the head-0
            # Q/K accumulation chains run paced by chunk arrivals instead of
            # waiting for one big transfer.
            nc.sync.dma_start(wq_sb[0][:], wq_d[0])
            for uc in range(UC):
                nc.sync.dma_start(x_ch[0][uc][:, 0:NH], xv[0, :, uc, 0:NH])
            nc.sync.dma_start(wk_sb[0][:], wk_d[0])
            nc.sync.dma_start(wv_sb[0][:], wv_d[0])
            nc.sync.dma_start(mq_sb[0][:, 0, :], mqv[0, :, 0, :])
            for uc in range(UC):
                nc.sync.dma_start(x_ch[0][uc][:, NH:S], xv[0, :, uc, NH:S])
                if uc % 2 == 1:
                    kc = 1 + uc // 2
                    nc.sync.dma_start(mq_sb[0][:, kc, :], mqv[0, :, kc, :])
            nc.sync.dma_start(wq_sb[1][:], wq_d[1])
            nc.sync.dma_start(wk_sb[1][:], wk_d[1])
            nc.sync.dma_start(wv_sb[1][:], wv_d[1])
            for kc in range(4, KT):
                nc.sync.dma_start(mq_sb[0][:, kc, :], mqv[0, :, kc, :])
            for h in range(2, H):
                nc.sync.dma_start(wq_sb[h][:], wq_d[h])
                nc.sync.dma_start(wk_sb[h][:], wk_d[h])
                nc.sync.dma_start(wv_sb[h][:], wv_d[h])
            wo_sb = sb.tile([P, UC, U], F16, tag="wo")
            nc.sync.dma_start(wo_sb[:], wo_d[:])
            if BPC > 1:
                for uc in range(UC):
                    nc.sync.dma_start(x_ch[1][uc][:], xv[1, :, uc, :])
                nc.sync.dma_start(mq_sb[1][:], mqv[1])

            def emit_qkv(b, h):
                qp = sb.tile([P, S], F16, tag="qp", bufs=2, name=f"qp{b}_{h}")
                ks = sb.tile([P, S], F16, tag="ks", bufs=2, name=f"ks{b}_{h}")
                vt = sb.tile([P, KT, C], F16, tag="vt", bufs=2, name=f"vt{b}_{h}")
                for half in range(2):
                    sl = slice(half * NH, (half + 1) * NH)
                    acc = ps.tile([P, NH], F32, tag="qk", bufs=4, name=f"accq{b}_{h}")
                    for uc in range(UC):
                        nc.tensor.matmul(
                            acc[:],
                            wq_sb[h][:, uc, :],
                            x_ch[b][uc][:, sl],
                            start=(uc == 0),
                            stop=(uc == UC - 1),
                        )
                    nc.scalar.copy(qp[:, sl], acc[:])
                    acc = ps.tile([P, NH], F32, tag="qk", bufs=4, name=f"acck{b}_{h}")
                    for uc in range(UC):
                        nc.tensor.matmul(
                            acc[:],
                            wk_sb[h][:, uc, :],
                            x_ch[b][uc][:, sl],
                            start=(uc == 0),
                            stop=(uc == UC - 1),
                        )
                    nc.scalar.copy(ks[:, sl], acc[:])
                    # V^T directly: stationary = x block, moving = wv
                    vtp = ps.tile([P, NH], F32, tag="qk", bufs=4, name=f"vtp{b}_{h}")
                    for j in range(4):
                        kc = half * 4 + j
                        for uc in range(UC):
                            nc.tensor.matmul(
                                vtp[:, j * C : (j + 1) * C],
                                x_ch[b][uc][:, kc * P : (kc + 1) * P],
                                wv_sb[h][:, uc, :],
                                start=(uc == 0),
                                stop=(uc == UC - 1),
                            )
                    nc.scalar.copy(
                        vt[:, half * 4 : (half + 1) * 4, :],
                        vtp[:].rearrange("p (j c) -> p j c", c=C),
                    )
                return qp, ks, vt

            qkv_pre = None
            for b in range(BPC):
                cc = sb.tile([P, H, S], F16, tag="cc", bufs=2, name=f"cc{b}")
                for h in range(H):
                    if qkv_pre is not None and qkv_pre[0] == (b, h):
                        qp, ks, vt = qkv_pre[1]
                        qkv_pre = None
                    else:
                        qp, ks, vt = emit_qkv(b, h)
                    opens = {}
                    if b + 1 < BPC and h == H - 1:
                        # pre-emit next batch's head-0 QKV: independent work
                        # the scheduler can use to fill the last head's
                        # attention-tail stalls (AV waiting on the DVE STT
                        # backlog) at the batch boundary
                        qkv_pre = ((b + 1, 0), emit_qkv(b + 1, 0))


                    # logits (transposed) + fused relu*maskq + AV accumulation
                    ch0 = ps.tile([P, NH], F32, tag="ch", bufs=2)
                    ch1 = ps.tile([P, NH], F32, tag="ch", bufs=2)
                    for kc in range(KT):
                        for half, ch in ((0, ch0), (1, ch1)):
                            a_ps = ps.tile([P, NH], F32, tag="a", bufs=2)
                            nc.tensor.matmul(
                                a_ps[:],
                                ks[:, kc * P : (kc + 1) * P],
                                qp[:, half * NH : (half + 1) * NH],
                                start=True,
                                stop=True,
                            )
                            atf = sb.tile([P, NH], F16, tag="atf", bufs=4)
                            nc.vector.scalar_tensor_tensor(
                                atf[:],
                                a_ps[:],
                                0.0,
                                mq_sb[b][:, kc, half * NH : (half + 1) * NH],
                                op0=mybir.AluOpType.max,
                                op1=mybir.AluOpType.mult,
                            )
                            nc.tensor.matmul(
                                ch[:],
                                vt[:, kc, :],
                                atf[:],
                                start=(kc == 0),
                                stop=(kc == KT - 1),
                            )
                    nc.scalar.copy(cc[:, h, 0:NH], ch0[:])
                    nc.scalar.copy(cc[:, h, NH:S], ch1[:])

                # ---- output projection (weights already resident) ----
                for ot in range(UC):
                    for half in range(2):
                        od = out_d[
                            b,
                            ot * P : (ot + 1) * P,
                            half * NH : (half + 1) * NH,
                        ]
                        if b == BPC - 1 and ot == UC - 1 and half == 1:
                            # final tile: 4 column-group accumulations in
                            # separate PSUM tiles with interleaved copies, so
                            # after the last matmul only one 128-col copy and
                            # the single DMA remain
                            o_sb = sb.tile([P, NH], F32, tag="o_sb", bufs=3)
                            for j in range(4):
                                jsl = slice(j * P, (j + 1) * P)
                                op_j = ps.tile(
                                    [P, P], F32, tag="qk", bufs=4, name=f"opfin{j}"
                                )
                                for uc in range(UC):
                                    nc.tensor.matmul(
                                        op_j[:],
                                        wo_sb[:, uc, ot * P : (ot + 1) * P],
                                        cc[:, uc, half * NH + j * P : half * NH + (j + 1) * P],
                                        start=(uc == 0),
                                        stop=(uc == UC - 1),
                                    )
                                nc.scalar.copy(o_sb[:, jsl], op_j[:])
                                if j == 2:
                                    nc.sync.dma_start(
                                        od[:, 0 : 3 * P], o_sb[:, 0 : 3 * P]
                                    )
                            nc.scalar.dma_start(od[:, 3 * P : NH], o_sb[:, 3 * P : NH])
                        else:
                            if (ot, half) in opens:
                                # close the pre-opened partial accumulation
                                o_ps = opens.pop((ot, half))
                                nc.tensor.matmul(
                                    o_ps[:],
                                    wo_sb[:, UC - 1, ot * P : (ot + 1) * P],
                                    cc[:, UC - 1, half * NH : (half + 1) * NH],
                                    start=False,
                                    stop=True,
                                )
                            else:
                                o_ps = ps.tile([P, NH], F32, tag="qk", bufs=4)
                                for uc in range(UC):
                                    nc.tensor.matmul(
                                        o_ps[:],
                                        wo_sb[:, uc, ot * P : (ot + 1) * P],
                                        cc[:, uc, half * NH : (half + 1) * NH],
                                        start=(uc == 0),
                                        stop=(uc == UC - 1),
                                    )
                            o_sb = sb.tile([P, NH], F32, tag="o_sb", bufs=3)
                            nc.scalar.copy(o_sb[:], o_ps[:])
                            nc.sync.dma_start(od, o_sb[:])

    nc.compile()
    return nc


_NC_CACHE = None


def _get_nc():
    global _NC_CACHE
    if _NC_CACHE is None:
        _NC_CACHE = build()
    return _NC_CACHE


def kernel(x, mask, w_qkv, w_out):
    nc = _get_nc()
    x = np.asarray(x, dtype=np.float32)
    mask_b = np.asarray(mask).astype(bool)
    w_qkv = np.asarray(w_qkv, dtype=np.float32)
    w_out = np.asarray(w_out, dtype=np.float32)

    # maskq[b,k,q] = mask[b,q,k] * scale / max(valid_count[b,q], 1)
    m = mask_b.sum(axis=2).astype(np.float32)  # [B, S]
    qs = SCALE / np.maximum(m, 1.0)
    maskq = mask_b.astype(np.float32) * qs[:, :, None]  # [B, q, k]
    mq = (
        np.ascontiguousarray(maskq.transpose(0, 2, 1))
        .astype(np.float16)
        .reshape(B, KT, P, S)
    )
    x16 = x.astype(np.float16).reshape(B, UC, P, S)

    wqkvT = np.ascontiguousarray(w_qkv.T).astype(np.float16)  # [U, 3U]
    packs = []
    for i in range(3):
        w_i = wqkvT[:, i * U : (i + 1) * U]  # [U, U]
        packs.append(
            np.ascontiguousarray(
                w_i.reshape(UC, P, H, C).transpose(2, 1, 0, 3)
            )  # [H, P, UC, C]
        )
    wq, wk, wv = packs
    wo = np.ascontiguousarray(
        w_out.T.astype(np.float16).reshape(UC, P, U).transpose(1, 0, 2)
    )  # [P, UC, U]

    in_maps = []
    for c in range(NCORES):
        in_maps.append(
            {
                "x16": np.ascontiguousarray(x16[c * BPC : (c + 1) * BPC]),
                "maskq": np.ascontiguousarray(mq[c * BPC : (c + 1) * BPC]),
                "wq": wq,
                "wk": wk,
                "wv": wv,
                "wo": wo,
            }
        )
    res = run_bass_kernel_spmd(nc, in_maps, list(range(NCORES)))
    out = np.concatenate([res.results[c]["out"] for c in range(NCORES)], axis=0)
    return out


# revision 42
# speedup vs baseline: 1.0102x; 1.0102x over previous
"""Multi-head attention (relu + valid-key-count normalization) on 8 TRN2 cores.

Strategy: data-parallel over batch (B=16 -> 2 per core), no collectives.
All matmul operands are float16 (full PE rate; 11-bit mantissa keeps the
end-to-end rel err ~1e-3 against the 2e-2 gate).

Key transformations vs the v1 baseline (367.7us):
- The reference normalization collapses to
      A_final[q,k] = relu(A[q,k]) * mask[q,k] * scale / max(m[q],1),
  and the whole multiplicative factor maskq[k,q] = mask[q,k]*scale/max(m,1)
  is precomputed ON HOST as an f16 tensor (transposed to match the
  k-on-partitions logits layout). This removes the on-device mask casts,
  mask transposes (8,192 PE cyc/batch), the m[q] accumulation, the qs
  broadcast chain, and the per-head Q'-scale multiply. relu+mask+norm is
  a single DVE scalar_tensor_tensor per logits tile, straight from PSUM.
- V^T is computed directly on the PE as x-stationary matmuls
  (V^T[k,c] = sum_u x[u,k] wv[u,c]) instead of V followed by PE
  transposes: same matmul cycles, minus 8,192 transpose cyc/batch.
- Weights are loaded once per core (f16, host-packed per head so every
  DMA is a full-contiguous descriptor) and stay resident in SBUF for
  both batches; x/mask DMAs are f16 (half the bytes of v1).
- All loads ride one DMA queue (SP) in explicit priority order (x half 0
  + head-0 weights first); out stores ride SP too so the ACT sequencer
  only does PSUM->SBUF copies. A chain of warm-up transposes fed by a
  DVE memset keeps the PE busy from ~1us until the first real matmul —
  otherwise the p-state ramp restarts after the prologue idle and the
  first ~6us of real matmuls run at 0.65-1.2GHz instead of 2.4.
- Batch b+1's head-0 QKV is emitted before batch b's last-head
  attention as scheduler filler for the DVE STT backlog; the final
  output tile is computed as 4 column-group accumulations so the drain
  tail ends on one small copy + store.

Timeline: 337.9us = 6.5us DMA-bound prologue (warm-up covered) +
326.9us PE busy (786,432 cycles @2.4GHz = the f16 matmul floor) +
~0.7us scheduling gaps + 4.0us store/drain tail. Measured rel err vs
the fp64-ish jax reference: ~7e-4 (gate 2e-2).
"""
import sys

sys.path.insert(0, "/opt/trn_rl_repo")

import numpy as np

import concourse.bacc as bacc
import concourse.mybir as mybir
import concourse.tile as tile
from concourse.bass_utils import run_bass_kernel_spmd

B, U, S, H, C = 16, 1024, 1024, 8, 128
NCORES = 8
BPC = B // NCORES  # batches per core
SCALE = float(1.0 / np.sqrt(np.float32(C)))
P = 128  # partitions
UC = U // P  # u chunks
QT = S // P  # q tiles
KT = S // P  # k tiles
NH = 512  # matmul free dim (one PSUM bank of f32)
NWARM = 34  # p-state warm-up transposes: bridge PE busy ~1us -> first matmul

F32 = mybir.dt.float32
F16 = mybir.dt.float16


def build():
    nc = bacc.Bacc()
    x_d = nc.dram_tensor("x16", [BPC, UC, P, S], F16, kind="ExternalInput")
    mq_d = nc.dram_tensor("maskq", [BPC, KT, P, S], F16, kind="ExternalInput")
    wq_d = nc.dram_tensor("wq", [H, P, UC, C], F16, kind="ExternalInput")
    wk_d = nc.dram_tensor("wk", [H, P, UC, C], F16, kind="ExternalInput")
    wv_d = nc.dram_tensor("wv", [H, P, UC, C], F16, kind="ExternalInput")
    wo_d = nc.dram_tensor("wo", [P, UC, U], F16, kind="ExternalInput")
    out_d = nc.dram_tensor("out", [BPC, U, S], F32, kind="ExternalOutput")

    xv = x_d[:].rearrange("b u p s -> b p u s")
    mqv = mq_d[:].rearrange("b k p s -> b p k s")

    with tile.TileContext(nc) as tc:
        with (
            tc.tile_pool(name="sb", bufs=1) as sb,
            tc.tile_pool(name="ps", bufs=1, space="PSUM") as ps,
        ):
            # warm-up: a DVE memset (no DMA dependency) feeds a chain of PE
            # transposes that keep the PE continuously busy from ~1us until
            # the first real matmul — otherwise the p-state ramp restarts
            # after the prologue idle and the first ~6us of real matmuls run
            # at 0.65-1.2GHz instead of 2.4GHz.
            wsrc = sb.tile([P, P], F16, tag="wsrc")
            nc.vector.memset(wsrc[:], 0.0)
            warm = ps.tile([P, P], F16, tag="a", bufs=2, name="warm")
            for _ in range(NWARM):
                nc.tensor.transpose(warm[:], wsrc[:], wsrc[:])

            # resident weights; head 0 first so QKV can start ASAP
            wq_sb = [
                sb.tile([P, UC, C], F16, tag=f"wq{h}", name=f"wq_sb{h}")
                for h in range(H)
            ]
            wk_sb = [
                sb.tile([P, UC, C], F16, tag=f"wk{h}", name=f"wk_sb{h}")
                for h in range(H)
            ]
            wv_sb = [
                sb.tile([P, UC, C], F16, tag=f"wv{h}", name=f"wv_sb{h}")
                for h in range(H)
            ]
            x_sb = [
                sb.tile([P, UC, S], F16, tag="x", bufs=2, name=f"x{b}")
                for b in range(BPC)
            ]
            mq_sb = [
                sb.tile([P, KT, S], F16, tag="mq", bufs=2, name=f"mq{b}")
                for b in range(BPC)
            ]
            # ALL loads on the SP queue in strict priority order — the DMA
            # engines device is serialized, so transfer order IS this order.
            # batch-0 x in column halves: the first QKV matmuls need only
            # half 0 of every uc chunk.
            nc.sync.dma_start(wq_sb[0][:], wq_d[0])
            nc.sync.dma_start(x_sb[0][:, 0:3, 0:NH], xv[0, :, 0:3, 0:NH])
            nc.sync.dma_start(x_sb[0][:, 3:6, 0:NH], xv[0, :, 3:6, 0:NH])
            nc.sync.dma_start(x_sb[0][:, 6:UC, 0:NH], xv[0, :, 6:UC, 0:NH])
            nc.sync.dma_start(wk_sb[0][:], wk_d[0])
            nc.sync.dma_start(wv_sb[0][:], wv_d[0])
            nc.sync.dma_start(x_sb[0][:, :, NH:S], xv[0, :, :, NH:S])
            for kc in range(4):
                nc.sync.dma_start(mq_sb[0][:, kc, :], mqv[0, :, kc, :])
            nc.sync.dma_start(wq_sb[1][:], wq_d[1])
            nc.sync.dma_start(wk_sb[1][:], wk_d[1])
            nc.sync.dma_start(wv_sb[1][:], wv_d[1])
            for kc in range(4, KT):
                nc.sync.dma_start(mq_sb[0][:, kc, :], mqv[0, :, kc, :])
            for h in range(2, H):
                nc.sync.dma_start(wq_sb[h][:], wq_d[h])
                nc.sync.dma_start(wk_sb[h][:], wk_d[h])
                nc.sync.dma_start(wv_sb[h][:], wv_d[h])
            wo_sb = sb.tile([P, UC, U], F16, tag="wo")
            nc.sync.dma_start(wo_sb[:], wo_d[:])
            if BPC > 1:
                nc.sync.dma_start(x_sb[1][:], xv[1])
                nc.sync.dma_start(mq_sb[1][:], mqv[1])

            def emit_qkv(b, h):
                qp = sb.tile([P, S], F16, tag="qp", bufs=2, name=f"qp{b}_{h}")
                ks = sb.tile([P, S], F16, tag="ks", bufs=2, name=f"ks{b}_{h}")
                vt = sb.tile([P, KT, C], F16, tag="vt", bufs=2, name=f"vt{b}_{h}")
                for half in range(2):
                    sl = slice(half * NH, (half + 1) * NH)
                    acc = ps.tile([P, NH], F32, tag="qk", bufs=4, name=f"accq{b}_{h}")
                    for uc in range(UC):
                        nc.tensor.matmul(
                            acc[:],
                            wq_sb[h][:, uc, :],
                            x_sb[b][:, uc, sl],
                            start=(uc == 0),
                            stop=(uc == UC - 1),
                        )
                    nc.scalar.copy(qp[:, sl], acc[:])
                    acc = ps.tile([P, NH], F32, tag="qk", bufs=4, name=f"acck{b}_{h}")
                    for uc in range(UC):
                        nc.tensor.matmul(
                            acc[:],
                            wk_sb[h][:, uc, :],
                            x_sb[b][:, uc, sl],
                            start=(uc == 0),
                            stop=(uc == UC - 1),
                        )
                    nc.scalar.copy(ks[:, sl], acc[:])
                    # V^T directly: stationary = x block, moving = wv
                    vtp = ps.tile([P, NH], F32, tag="qk", bufs=4, name=f"vtp{b}_{h}")
                    for j in range(4):
                        kc = half * 4 + j
                        for uc in range(UC):
                            nc.tensor.matmul(
                                vtp[:, j * C : (j + 1) * C],
                                x_sb[b][:, uc, kc * P : (kc + 1) * P],
                                wv_sb[h][:, uc, :],
                                start=(uc == 0),
                                stop=(uc == UC - 1),
                            )
                    nc.scalar.copy(
                        vt[:, half * 4 : (half + 1) * 4, :],
                        vtp[:].rearrange("p (j c) -> p j c", c=C),
                    )
                return qp, ks, vt

            qkv_pre = None
            for b in range(BPC):
                cc = sb.tile([P, H, S], F16, tag="cc", bufs=2, name=f"cc{b}")
                for h in range(H):
                    if qkv_pre is not None and qkv_pre[0] == (b, h):
                        qp, ks, vt = qkv_pre[1]
                        qkv_pre = None
                    else:
                        qp, ks, vt = emit_qkv(b, h)
                    if b + 1 < BPC and h == H - 1:
                        # pre-emit next batch's head-0 QKV: independent work
                        # the scheduler can use to fill the last head's
                        # attention-tail stalls (AV waiting on the DVE STT
                        # backlog) at the batch boundary
                        qkv_pre = ((b + 1, 0), emit_qkv(b + 1, 0))


                    # logits (transposed) + fused relu*maskq + AV accumulation
                    ch0 = ps.tile([P, NH], F32, tag="ch", bufs=2)
                    ch1 = ps.tile([P, NH], F32, tag="ch", bufs=2)
                    for kc in range(KT):
                        for half, ch in ((0, ch0), (1, ch1)):
                            a_ps = ps.tile([P, NH], F32, tag="a", bufs=2)
                            nc.tensor.matmul(
                                a_ps[:],
                                ks[:, kc * P : (kc + 1) * P],
                                qp[:, half * NH : (half + 1) * NH],
                                start=True,
                                stop=True,
                            )
                            atf = sb.tile([P, NH], F16, tag="atf", bufs=4)
                            nc.vector.scalar_tensor_tensor(
                                atf[:],
                                a_ps[:],
                                0.0,
                                mq_sb[b][:, kc, half * NH : (half + 1) * NH],
                                op0=mybir.AluOpType.max,
                                op1=mybir.AluOpType.mult,
                            )
                            nc.tensor.matmul(
                                ch[:],
                                vt[:, kc, :],
                                atf[:],
                                start=(kc == 0),
                                stop=(kc == KT - 1),
                            )
                    nc.scalar.copy(cc[:, h, 0:NH], ch0[:])
                    if h == H - 1:
                        # last head: half-1 copy on DVE (free after its final
                        # STT) so both copies run in parallel and the
                        # out-proj's uc=7 matmuls aren't serialized behind ACT
                        nc.vector.tensor_copy(cc[:, h, NH:S], ch1[:])
                    else:
                        nc.scalar.copy(cc[:, h, NH:S], ch1[:])

                # ---- output projection (weights already resident) ----
                for ot in range(UC):
                    for half in range(2):
                        od = out_d[
                            b,
                            ot * P : (ot + 1) * P,
                            half * NH : (half + 1) * NH,
                        ]
                        if b == BPC - 1 and ot == UC - 1 and half == 1:
                            # final tile: 4 column-group accumulations in
                            # separate PSUM tiles with interleaved copies, so
                            # after the last matmul only one 128-col copy and
                            # the single DMA remain
                            o_sb = sb.tile([P, NH], F32, tag="o_sb", bufs=3)
                            for j in range(4):
                                jsl = slice(j * P, (j + 1) * P)
                                op_j = ps.tile(
                                    [P, P], F32, tag="qk", bufs=4, name=f"opfin{j}"
                                )
                                for uc in range(UC):
                                    nc.tensor.matmul(
                                        op_j[:],
                                        wo_sb[:, uc, ot * P : (ot + 1) * P],
                                        cc[:, uc, half * NH + j * P : half * NH + (j + 1) * P],
                                        start=(uc == 0),
                                        stop=(uc == UC - 1),
                                    )
                                nc.scalar.copy(o_sb[:, jsl], op_j[:])
                                if j == 2:
                                    nc.sync.dma_start(
                                        od[:, 0 : 3 * P], o_sb[:, 0 : 3 * P]
                                    )
                            # last chunk alone: the drain tail ends on a
                            # 128-col transfer instead of a full 512
                            nc.scalar.dma_start(od[:, 3 * P : NH], o_sb[:, 3 * P : NH])
                        else:
                            o_ps = ps.tile([P, NH], F32, tag="qk", bufs=4)
                            for uc in range(UC):
                                nc.tensor.matmul(
                                    o_ps[:],
                                    wo_sb[:, uc, ot * P : (ot + 1) * P],
                                    cc[:, uc, half * NH : (half + 1) * NH],
                                    start=(uc == 0),
                                    stop=(uc == UC - 1),
                                )
                            o_sb = sb.tile([P, NH], F32, tag="o_sb", bufs=3)
                            nc.scalar.copy(o_sb[:], o_ps[:])
                            nc.sync.dma_start(od, o_sb[:])

    nc.compile()
    return nc


_NC_CACHE = None


def _get_nc():
    global _NC_CACHE
    if _NC_CACHE is None:
        _NC_CACHE = build()
    return _NC_CACHE


def kernel(x, mask, w_qkv, w_out):
    nc = _get_nc()
    x = np.asarray(x, dtype=np.float32)
    mask_b = np.asarray(mask).astype(bool)
    w_qkv = np.asarray(w_qkv, dtype=np.float32)
    w_out = np.asarray(w_out, dtype=np.float32)

    # maskq[b,k,q] = mask[b,q,k] * scale / max(valid_count[b,q], 1)
    m = mask_b.sum(axis=2).astype(np.float32)  # [B, S]
    qs = SCALE / np.maximum(m, 1.0)
    maskq = mask_b.astype(np.float32) * qs[:, :, None]  # [B, q, k]
    mq = (
        np.ascontiguousarray(maskq.transpose(0, 2, 1))
        .astype(np.float16)
        .reshape(B, KT, P, S)
    )
    x16 = x.astype(np.float16).reshape(B, UC, P, S)

    wqkvT = np.ascontiguousarray(w_qkv.T).astype(np.float16)  # [U, 3U]
    packs = []
    for i in range(3):
        w_i = wqkvT[:, i * U : (i + 1) * U]  # [U, U]
        packs.append(
            np.ascontiguousarray(
                w_i.reshape(UC, P, H, C).transpose(2, 1, 0, 3)
            )  # [H, P, UC, C]
        )
    wq, wk, wv = packs
    wo = np.ascontiguousarray(
        w_out.T.astype(np.float16).reshape(UC, P, U).transpose(1, 0, 2)
    )  # [P, UC, U]

    in_maps = []
    for c in range(NCORES):
        in_maps.append(
            {
                "x16": np.ascontiguousarray(x16[c * BPC : (c + 1) * BPC]),
                "maskq": np.ascontiguousarray(mq[c * BPC : (c + 1) * BPC]),
                "wq": wq,
                "wk": wk,
                "wv": wv,
                "wo": wo,
            }
        )
    res = run_bass_kernel_spmd(nc, in_maps, list(range(NCORES)))
    out = np.concatenate([res.results[c]["out"] for c in range(NCORES)], axis=0)
    return out
